# revision 35
# baseline (speedup 1.0000x reference)
"""Causal single-head attention (B=4, T=4096, D=1024) on 8 trn2 NeuronCores.

Sharding: 2 cores per batch element, split by key-block PARITY (flash-style):
  core = 2*b + p ; p in {0,1}
  Each core computes, for ALL 4096 queries of batch b, the partial
  (unnormalized) attention output over its 16 key blocks {128*(2u+p)} and the
  partial softmax row-sums. Host merges: O = (O_0 + O_1) / (rs_0 + rs_1).

v3 design:
  Key-side M-fold: scores = q.k^T = x Wq^T Wk x̃^T = x (x̃ M^T)^T with
    M = Wq^T Wk precomputed on host. BOTH projections (k' = x̃ M^T and
    V = x̃ Wv^T) now apply only to the core's own 2048 keys, so no work is
    duplicated across the pair and the query side streams raw x chunks
    straight from DRAM. Per-core matmul work: 26.9 GFLOP (the causal
    minimum for this sharding).
  bf16 matmul operands throughout (PSUM accumulation fp32; ~4e-3 rel err
    vs the fp32 reference, tolerance 2e-2). bf16 stationary ldweights
    pipeline behind matmuls, unlike fp32's.
  Output drains split across the Vector and Scalar engines so the final
    chunk's drain isn't serialized on one engine.

Per-core phases (identical program, data-only differences):
  kproj: k'^T = M^T-blocks @ x̃^T   (SBUF-resident, 32 KiB/partition)
  vproj: V    = x̃-blocks @ Wv^T    (+ones cols for row-sums)
  attn(j), j = 15..0: per q-chunk of 256 columns:
    S^T = k'^T-blk^T @ x^T-chunk (PSUM), P^T = exp(S^T/32) (ACT),
    diagonal mask on last key block, O' += P^T-sub^T @ V-blk (PSUM accum),
    drain O'(+rs) -> DRAM two chunks behind, software-pipelined.
"""

import sys

sys.path.insert(0, "/opt/trn_rl_repo")

import numpy as np
import ml_dtypes
from contextlib import ExitStack

import concourse.tile as tile
from concourse import bacc, mybir
from concourse.bass_utils import run_bass_kernel_spmd

P = 128
D = 1024
T = 4096
B = 4
NDB = D // P  # 8 feature blocks
NCB = D // P  # 8 contraction blocks
NKB = 16  # key blocks per core (parity half of 32)
QC = 256  # query-chunk columns
NQC = T // QC  # 16
F32 = mybir.dt.float32
BF16 = mybir.dt.bfloat16
EXPSCALE = 1.0 / 32.0  # 1/sqrt(D)
EXP = mybir.ActivationFunctionType.Exp

_CACHED_NC = None
_LAST_RES = None


def _build_program():
    nc = bacc.Bacc("TRN2", target_bir_lowering=False, debug=False, num_devices=8)

    xT_d = nc.dram_tensor("xT", [D, T], BF16, kind="ExternalInput").ap()
    xTk_d = nc.dram_tensor("xTk", [D, T // 2], BF16, kind="ExternalInput").ap()
    m_d = nc.dram_tensor("MT", [D, D], BF16, kind="ExternalInput").ap()  # M^T
    wv_d = nc.dram_tensor("WvT", [D, D], BF16, kind="ExternalInput").ap()
    mask_d = nc.dram_tensor("mask", [P, QC], F32, kind="ExternalInput").ap()
    ones4_d = nc.dram_tensor("ones4", [P, 4], BF16, kind="ExternalInput").ap()
    o_d = nc.dram_tensor("O", [T, D], F32, kind="ExternalOutput").ap()
    rs_d = nc.dram_tensor("rs", [T, 1], F32, kind="ExternalOutput").ap()

    xT_r = xT_d.rearrange("(a p) t -> p a t", p=P)  # [128, 8, 4096]
    xTk_r = xTk_d.rearrange("(a p) t -> p a t", p=P)  # [128, 8, 2048]
    m_r = m_d.rearrange("(a p) d -> p a d", p=P)  # [128, 8, 1024]
    wv_r = wv_d.rearrange("(a p) d -> p a d", p=P)

    with tile.TileContext(nc) as tc, ExitStack() as ctx:
        kv = ctx.enter_context(tc.tile_pool(name="kv", bufs=1))
        xp = ctx.enter_context(tc.tile_pool(name="xp", bufs=4))
        wp = ctx.enter_context(tc.tile_pool(name="wp", bufs=2))
        pp = ctx.enter_context(tc.tile_pool(name="pp", bufs=4))
        stg = ctx.enter_context(tc.tile_pool(name="stg", bufs=6))
        psum = ctx.enter_context(tc.tile_pool(name="psum", bufs=1, space="PSUM"))

        mask_t = kv.tile([P, QC], F32, tag="mask")
        mT_t = kv.tile([P, NCB, D], BF16, tag="mT")  # M^T, 16 KiB/part
        kt_t = kv.tile([P, NCB, T // 2], BF16, tag="kt")  # x̃^T, 32 KiB
        kpT_t = kv.tile([P, NCB, T // 2], BF16, tag="kpT")  # k'^T, 32 KiB
        v_t = kv.tile([P, NKB, D + 4], BF16, tag="vt")  # 32.1 KiB

        # ---- startup DMAs, ordered so kproj can begin after ~1.25 MB ----
        nc.sync.dma_start(mT_t[:, 0, :], m_r[:, 0, :])
        nc.sync.dma_start(kt_t[:, :, 0:512], xTk_r[:, :, 0:512])
        for cb in range(1, NCB):
            nc.sync.dma_start(mT_t[:, cb, :], m_r[:, cb, :])
        for g in range(1, 4):
            nc.sync.dma_start(
                kt_t[:, :, g * 512 : (g + 1) * 512],
                xTk_r[:, :, g * 512 : (g + 1) * 512],
            )
        wvs = []
        for vc in range(2):
            wv = wp.tile([P, NCB, 512], BF16, tag="wv", name=f"wv{vc}")
            nc.sync.dma_start(wv[:], wv_r[:, :, vc * 512 : (vc + 1) * 512])
            wvs.append(wv)
        xqs = {}

        def fetch(j):
            if 0 <= j and j not in xqs:
                t = xp.tile([P, NCB, QC], BF16, tag="x", name=f"xq{j}")
                nc.sync.dma_start(t[:], xT_r[:, :, j * QC : (j + 1) * QC])
                xqs[j] = t

        ORDER = list(range(14, -1, -1)) + [15]  # end on a big chunk: its score
        # blocks cover the small chunks' drains; only the terminal drain is bare
        for j in ORDER[:3]:
            fetch(j)
        nc.sync.dma_start(mask_t[:], mask_d[:])

        ROT = ["b0", "b1", "b2", "b3", "s0", "s1"]
        rot = 0

        # ---- k' projection: k'^T[i, s] = sum_j M[i,j] x̃^T[j, s] ----
        for g in range(4):
            for ig in range(NDB):
                ps = psum.tile([P, 512], F32, tag=ROT[rot % 6], name=f"kps{g}_{ig}")
                rot += 1
                for cb in range(NCB):
                    nc.tensor.matmul(
                        ps[:],
                        mT_t[:, cb, ig * P : (ig + 1) * P],
                        kt_t[:, cb, g * 512 : (g + 1) * 512],
                        start=(cb == 0),
                        stop=(cb == NCB - 1),
                    )
                nc.vector.tensor_copy(kpT_t[:, ig, g * 512 : (g + 1) * 512], ps[:])

        # ---- V projection ----
        for g in range(4):
            for vc in range(2):
                for kb in range(4 * g, 4 * g + 4):
                    ps = psum.tile([P, 512], F32, tag=ROT[rot % 6], name=f"vps{vc}_{kb}")
                    rot += 1
                    for cb in range(NCB):
                        nc.tensor.matmul(
                            ps[:],
                            kt_t[:, cb, kb * P : (kb + 1) * P],
                            wvs[vc][:, cb, :],
                            start=(cb == 0),
                            stop=(cb == NCB - 1),
                        )
                    nc.vector.tensor_copy(v_t[:, kb, vc * 512 : (vc + 1) * 512], ps[:])
        for kb in range(NKB):
            nc.sync.dma_start(v_t[:, kb, D : D + 4], ones4_d[:])

        # ---- attention, software-pipelined ----
        prev = None  # (acc dict, j) pending drain

        def drain(d_acc, d_j, q=None):
            dma = nc.gpsimd.dma_start if q is None else nc.sync.dma_start
            for sub in range(2):
                row = d_j * QC + sub * P
                ot0 = stg.tile([P, 512], F32, tag="stage", name=f"ot0_{d_j}_{sub}")
                nc.vector.tensor_copy(ot0[:], d_acc[sub, 0][:])
                ot1 = stg.tile([P, 512], F32, tag="stage", name=f"ot1_{d_j}_{sub}")
                nc.vector.tensor_copy(ot1[:], d_acc[sub, 1][:])
                rt = stg.tile([P, 1], F32, tag="rt", name=f"rt{d_j}_{sub}")
                nc.scalar.copy(rt[:], d_acc[sub, 2][:, 0:1])
                dma(o_d[row : row + P, 0:512], ot0[:])
                dma(o_d[row : row + P, 512:1024], ot1[:])
                dma(rs_d[row : row + P, :], rt[:])

        def attn(j, pre):
            nonlocal prev
            if pre is not None:
                fetch(pre)
            xq = xqs.pop(j)
            acc = {}
            for sub in range(2):
                acc[sub, 0] = psum.tile([P, 512], F32, tag=f"b{2 * sub}", name=f"a0_{j}_{sub}")
                acc[sub, 1] = psum.tile([P, 512], F32, tag=f"b{2 * sub + 1}", name=f"a1_{j}_{sub}")
                acc[sub, 2] = psum.tile([P, 4], F32, tag=f"r{sub}", name=f"a2_{j}_{sub}")

            def av(u, pt_t, first, last):
                for sub in range(2):
                    lhs = pt_t[:, sub * P : (sub + 1) * P]
                    nc.tensor.matmul(
                        acc[sub, 0][:], lhs, v_t[:, u, 0:512],
                        start=first, stop=last, skip_group_check=True,
                    )
                    nc.tensor.matmul(
                        acc[sub, 1][:], lhs, v_t[:, u, 512:1024],
                        start=first, stop=last, skip_group_check=True,
                    )
                    nc.tensor.matmul(
                        acc[sub, 2][:], lhs, v_t[:, u, D : D + 4],
                        start=first, stop=last, skip_group_check=True,
                    )

            pts = {}
            for u in range(j + 1):
                st = psum.tile([P, QC], F32, tag=f"s{u % 2}", name=f"st{j}_{u}")
                for db in range(NDB):
                    nc.tensor.matmul(
                        st[:],
                        kpT_t[:, db, u * P : (u + 1) * P],
                        xq[:, db, :],
                        start=(db == 0),
                        stop=(db == NDB - 1),
                    )
                if u == j:
                    nc.vector.tensor_add(st[:], st[:], mask_t[:])
                pt = pp.tile([P, QC], BF16, tag="pt", name=f"pt{j}_{u}")
                nc.scalar.activation(pt[:], st[:], EXP, scale=EXPSCALE)
                pts[u] = pt
                if u == 2 and prev is not None:
                    drain(*prev)
                    prev = None
                if u >= 2:
                    av(u - 2, pts.pop(u - 2), first=(u == 2), last=False)
            if prev is not None:  # j in {0, 1}
                drain(*prev)
                prev = None
            if j >= 1:
                av(j - 1, pts.pop(j - 1), first=(j == 1), last=False)
            av(j, pts.pop(j), first=(j == 0), last=True)
            prev = (acc, j)

        for i, j in enumerate(ORDER):
            attn(j, ORDER[i + 3] if i + 3 < NQC else None)
        drain(*prev, q="sync")

    nc.finalize()
    return nc


def _get_program():
    global _CACHED_NC
    if _CACHED_NC is None:
        _CACHED_NC = _build_program()
    return _CACHED_NC


def _masks():
    neg = np.float32(-1e30)
    tri = np.where(np.triu(np.ones((P, P), dtype=bool)), np.float32(0), neg)
    keep = np.zeros((P, P), dtype=np.float32)
    drop = np.full((P, P), neg, dtype=np.float32)
    return (
        np.ascontiguousarray(np.concatenate([tri, keep], axis=1)),  # even core
        np.ascontiguousarray(np.concatenate([drop, tri], axis=1)),  # odd core
    )


def kernel(x, Wq, Wk, Wv):
    out, _ = _run(x, Wq, Wk, Wv, trace=False)
    return out


def _run(x, Wq, Wk, Wv, trace=False, keep_res=False):
    BF = ml_dtypes.bfloat16
    x = np.asarray(x, dtype=np.float32)
    M = (np.asarray(Wq, np.float64).T @ np.asarray(Wk, np.float64)).astype(np.float32)
    MT_bf = np.ascontiguousarray(M.T.astype(BF))  # [j, i] layout for k'proj
    WvT_bf = np.ascontiguousarray(np.asarray(Wv, np.float32).T.astype(BF))
    m_even, m_odd = _masks()
    ones4 = np.ascontiguousarray(
        np.repeat(np.array([[1.0, 0.0, 0.0, 0.0]], dtype=np.float32), P, axis=0).astype(BF)
    )

    nc = _get_program()
    in_maps = []
    for core in range(8):
        b, p = core // 2, core % 2
        xT = np.ascontiguousarray(x[b].T.astype(BF))  # [D, T]
        xTk = np.ascontiguousarray(
            xT.reshape(D, T // P, P)[:, p::2, :].reshape(D, T // 2)
        )
        in_maps.append(
            {
                "xT": xT,
                "xTk": xTk,
                "MT": MT_bf,
                "WvT": WvT_bf,
                "mask": m_even if p == 0 else m_odd,
                "ones4": ones4,
            }
        )

    res = run_bass_kernel_spmd(nc, in_maps, core_ids=list(range(8)), trace=trace)
    if keep_res:
        global _LAST_RES
        _LAST_RES = res
    out = np.empty((B, T, D), dtype=np.float32)
    for b in range(B):
        O0, rs0 = res.results[2 * b]["O"], res.results[2 * b]["rs"]
        O1, rs1 = res.results[2 * b + 1]["O"], res.results[2 * b + 1]["rs"]
        out[b] = (O0 + O1) / (rs0 + rs1)
    return out, res.exec_time_ns
